# revision 4
# baseline (speedup 1.0000x reference)
"""Multi-head attention (B=2, N=4096, C=512, H=8, d=64) on 8 Trainium2 NeuronCores.

Sharding: core c handles batch b = c//4 and heads {2*(c%4), 2*(c%4)+1}.
Each core computes its 2 heads' attention plus a partial output projection
(contraction over its 128 rows of W_proj); the host gather sums the 4
partials per batch (bias is added on the p==0 core of each batch).

On-device dataflow (transposed-scores formulation, no on-chip transposes):
  qT/kT [128=2*64 d-dims, 4096]  = W.T @ x.T      (x.T supplied by host)
  v_aug [128 n-chunk, 32*(65+65)] = x @ Wv with a ones column per head
  S^T[kidx, q] = kT.T_chunk @ qT  (two heads ride row-groups 0-1 / 2-3
                                   of the PE array concurrently, K=64 each)
  E = exp(S^T / 8)                (ScalarE, scale folded into the LUT affine)
  [out_unnorm^T; den] = v_aug.T @ E   (ones column makes row 64 the softmax
                                       denominator -- no extra pass)
  out^T = out_unnorm^T * (1/den)  (reciprocal + K=1 broadcast matmul)
  partial = out^T.T @ W_proj_slice + bias   (per-head K=64 contractions)
"""

import sys
import types

for _p in ("/opt/trn_rl_repo",):
    if _p not in sys.path:
        sys.path.insert(0, _p)

import numpy as np
import ml_dtypes
from contextlib import ExitStack

# antenv.axon_hooks shim: lets run_bass_kernel_spmd find the NTFF profiling
# hook when BASS_TRACE=1 (the agent image's antenv lacks this module).
import antenv  # noqa: F401

if "antenv.axon_hooks" not in sys.modules:
    _m = types.ModuleType("antenv.axon_hooks")
    _m._hook = None

    def _set_hook(h):
        _m._hook = h

    def _get_hook():
        return _m._hook

    _m.set_axon_ntff_profile_hook = _set_hook
    _m.get_axon_ntff_profile_hook = _get_hook
    sys.modules["antenv.axon_hooks"] = _m
    try:
        from trn_agent_boot.trn_boot import _ntff_profile_via_ctypes

        hook = _ntff_profile_via_ctypes("/opt/axon/libaxon_pjrt.so")
        if hook is not None:
            _set_hook(hook)
    except Exception:
        pass

import concourse.bass as bass  # noqa: E402
import concourse.tile as tile  # noqa: E402
from concourse import mybir, bacc  # noqa: E402
from concourse import bass_utils  # noqa: E402

# No bucket storage in this container; artifacts stay local.
bass_utils.upload_artifacts = lambda tmpdir: f"local://{tmpdir}"

B, N, C = 2, 4096, 512
H, D = 8, 64
N_CORES = 8
SCALE = D ** -0.5

BF16 = mybir.dt.bfloat16
F32 = mybir.dt.float32
AF = mybir.ActivationFunctionType
BFNP = ml_dtypes.bfloat16

NI = N // 128   # 32 kidx / n chunks
NJ = N // 1024  # 4 q blocks
VW = 2 * (D + 1)  # 130: per-n-chunk vaug block (2 heads x (64 v + 1 ones))


def build_nc():
    nc = bacc.Bacc("TRN2", target_bir_lowering=False, debug=False)

    xt = nc.dram_tensor("xt", [4, 128, N], BF16, kind="ExternalInput").ap()
    wq = nc.dram_tensor("wq", [128, 512], BF16, kind="ExternalInput").ap()
    wk = nc.dram_tensor("wk", [128, 512], BF16, kind="ExternalInput").ap()
    wv = nc.dram_tensor("wv", [128, 512], BF16, kind="ExternalInput").ap()
    wp = nc.dram_tensor("wp", [64, 1024], BF16, kind="ExternalInput").ap()
    bias = nc.dram_tensor("bias", [1, 512], BF16, kind="ExternalInput").ap()
    out = nc.dram_tensor("out", [N, C], F32, kind="ExternalOutput").ap()

    with tile.TileContext(nc) as tc:
        with ExitStack() as ctx:
            const = ctx.enter_context(tc.tile_pool(name="const", bufs=1))
            sb = ctx.enter_context(tc.tile_pool(name="sb", bufs=1))
            expp = ctx.enter_context(tc.tile_pool(name="expp", bufs=3))
            invp = ctx.enter_context(tc.tile_pool(name="invp", bufs=2))
            outp = ctx.enter_context(tc.tile_pool(name="outp", bufs=3))

            twq = const.tile([128, 512], BF16)
            nc.sync.dma_start(twq[:], wq[:])
            twk = const.tile([128, 512], BF16)
            nc.sync.dma_start(twk[:], wk[:])
            twv = const.tile([128, 512], BF16)
            nc.sync.dma_start(twv[:], wv[:])
            twp = const.tile([64, 1024], BF16)
            nc.sync.dma_start(twp[:], wp[:])
            tbias = const.tile([1, 512], BF16)
            nc.sync.dma_start(tbias[:], bias[:])
            tones = const.tile([1, 128], BF16)
            nc.gpsimd.memset(tones[:], 1.0)

            qT = sb.tile([128, N], BF16)
            kT = sb.tile([128, N], BF16)
            vaug = sb.tile([128, NI * VW], BF16)
            nc.gpsimd.memset(vaug[:], 1.0)
            outT0 = sb.tile([64, N], BF16)
            outT1 = sb.tile([64, N], BF16)
            outTs = (outT0, outT1)

            # ---- stage A: QKV projections -------------------------------
            with tc.tile_pool(name="xtp", bufs=1) as xtp, tc.tile_pool(
                name="psA", bufs=2, space="PSUM"
            ) as psA:
                xts = []
                for k in range(4):
                    t = xtp.tile([128, N], BF16, tag=f"xt{k}")
                    nc.sync.dma_start(t[:], xt[k])
                    xts.append(t)
                for j8 in range(8):
                    s = bass.ts(j8, 512)
                    for w, dst in ((twq, qT), (twk, kT)):
                        ps = psA.tile([128, 512], F32, tag="qk")
                        for k in range(4):
                            nc.tensor.matmul(
                                ps[:],
                                w[:, bass.ts(k, 128)],
                                xts[k][:, s],
                                start=(k == 0),
                                stop=(k == 3),
                            )
                        nc.vector.tensor_copy(dst[:, s], ps[:])
                for jj in range(NI):
                    ps = psA.tile([128, 128], F32, tag="v")
                    for k in range(4):
                        nc.tensor.matmul(
                            ps[:],
                            xts[k][:, bass.ts(jj, 128)],
                            twv[:, bass.ts(k, 128)],
                            start=(k == 0),
                            stop=(k == 3),
                        )
                    dst = vaug[:, jj * VW : (jj + 1) * VW].rearrange(
                        "p (h c) -> p h c", h=2
                    )[:, :, 0:D]
                    src = ps[:].rearrange("p (h c) -> p h c", h=2)
                    nc.vector.tensor_copy(dst, src)

            # ---- stage B: scores^T -> exp -> AV (+den), normalize -------
            # ---- stage C: partial projection + bias ---------------------
            with tc.tile_pool(name="psS", bufs=3, space="PSUM") as psS, tc.tile_pool(
                name="psAV", bufs=1, space="PSUM"
            ) as psAV:
                for j in range(8):
                    qs = bass.ts(j, 512)
                    avs = [
                        psAV.tile([65, 512], F32, tag=f"av{t}", name=f"av{t}")
                        for t in range(2)
                    ]  # one per head
                    for i in range(NI):
                        ks = bass.ts(i, 128)
                        pss = psS.tile([128, 1024], F32, tag="s")
                        nc.tensor.matmul(
                            pss[:, 0:512],
                            kT[0:64, ks],
                            qT[0:64, qs],
                            start=True,
                            stop=True,
                        )
                        nc.tensor.matmul(
                            pss[:, 512:1024],
                            kT[64:128, ks],
                            qT[64:128, qs],
                            start=True,
                            stop=True,
                        )
                        est = expp.tile([128, 1024], BF16, tag="e")
                        nc.scalar.activation(est[:], pss[:], AF.Exp, scale=SCALE)
                        for h in range(2):
                            va = vaug[:, i * VW + h * 65 : i * VW + (h + 1) * 65]
                            nc.tensor.matmul(
                                avs[h][:],
                                va,
                                est[:, bass.ts(h, 512)],
                                start=(i == 0),
                                stop=(i == NI - 1),
                            )
                    for h in range(2):
                        inv = invp.tile([1, 512], BF16, tag="inv")
                        with nc.allow_low_precision(reason="softmax 1/den in bf16"):
                            nc.vector.reciprocal(inv[:], avs[h][64:65, :])
                        psb = psS.tile([64, 512], F32, tag="s")
                        nc.tensor.matmul(
                            psb[:], tones[0:1, 0:64], inv[:], start=True, stop=True
                        )
                        sbb = invp.tile([64, 512], BF16, tag="sbb")
                        nc.vector.tensor_copy(sbb[:], psb[:])
                        nc.vector.tensor_mul(
                            outTs[h][:, qs], avs[h][0:64, :], sbb[:]
                        )
                    for jj in range(j * 4, (j + 1) * 4):
                        s = bass.ts(jj, 128)
                        pp = psS.tile([128, 512], F32, tag="s")
                        nc.tensor.matmul(
                            pp[:], outT0[:, s], twp[:, 0:512], start=True, stop=False
                        )
                        nc.tensor.matmul(
                            pp[:], outT1[:, s], twp[:, 512:1024], start=False, stop=False
                        )
                        nc.tensor.matmul(
                            pp[:], tones[0:1, :], tbias[:], start=False, stop=True
                        )
                        ot = outp.tile([128, 512], F32, tag="o")
                        nc.vector.tensor_copy(ot[:], pp[:])
                        nc.sync.dma_start(out[s, :], ot[:])

    nc.compile()
    return nc


def _pack_w(wslice):
    # [512, 128] -> SBUF image [128, 4*128] with C-chunk k at cols k*128..
    return np.ascontiguousarray(
        wslice.reshape(4, 128, 128).transpose(1, 0, 2).reshape(128, 512)
    ).astype(BFNP)


_NC_CACHE = None
LAST_RESULT = None


def kernel(x, W_qkv, W_proj, b_proj):
    global _NC_CACHE, LAST_RESULT
    x = np.asarray(x, dtype=np.float32)
    W_qkv = np.asarray(W_qkv, dtype=np.float32)
    W_proj = np.asarray(W_proj, dtype=np.float32)
    b_proj = np.asarray(b_proj, dtype=np.float32)

    if _NC_CACHE is None:
        _NC_CACHE = build_nc()
    nc = _NC_CACHE

    in_maps = []
    for c in range(N_CORES):
        b = c // 4
        h0 = 2 * (c % 4)
        xtb = np.ascontiguousarray(x[b].T).reshape(4, 128, N).astype(BFNP)
        wq = _pack_w(W_qkv[:, h0 * 64 : h0 * 64 + 128])
        wk = _pack_w(W_qkv[:, 512 + h0 * 64 : 512 + h0 * 64 + 128])
        wv = _pack_w(W_qkv[:, 1024 + h0 * 64 : 1024 + h0 * 64 + 128])
        wp = np.ascontiguousarray(
            np.concatenate(
                [
                    W_proj[h0 * 64 : (h0 + 1) * 64, :],
                    W_proj[(h0 + 1) * 64 : (h0 + 2) * 64, :],
                ],
                axis=1,
            )
        ).astype(BFNP)
        bias = (
            b_proj[None, :].astype(BFNP)
            if c % 4 == 0
            else np.zeros((1, 512), dtype=BFNP)
        )
        in_maps.append(
            {"xt": xtb, "wq": wq, "wk": wk, "wv": wv, "wp": wp, "bias": bias}
        )

    res = bass_utils.run_bass_kernel_spmd(
        nc, in_maps, core_ids=list(range(N_CORES))
    )
    LAST_RESULT = res

    out = np.zeros((B, N, C), dtype=np.float32)
    for c in range(N_CORES):
        out[c // 4] += res.results[c]["out"]
    return out


# revision 5
# speedup vs baseline: 1.1102x; 1.1102x over previous
"""Multi-head attention (B=2, N=4096, C=512, H=8, d=64) on 8 Trainium2 NeuronCores.

Sharding: core c handles batch b = c//4 and heads {2*(c%4), 2*(c%4)+1}.
Each core computes its 2 heads' attention plus a partial output projection
(contraction over its 128 rows of W_proj); the host gather sums the 4
partials per batch (bias is added on the p==0 core of each batch).

On-device dataflow (transposed-scores formulation, no on-chip transposes):
  qT/kT [128=2*64 d-dims, 4096]  = W.T @ x.T      (x.T supplied by host)
  v_aug [128 n-chunk, 32*(65+65)] = x @ Wv with a ones column per head
  S^T[kidx, q] = kT.T_chunk @ qT  (two heads ride row-groups 0-1 / 2-3
                                   of the PE array concurrently, K=64 each)
  E = exp(S^T / 8)                (ScalarE, scale folded into the LUT affine)
  [out_unnorm^T; den] = v_aug.T @ E   (ones column makes row 64 the softmax
                                       denominator -- no extra pass)
  out^T = out_unnorm^T * (1/den)  (reciprocal + K=1 broadcast matmul)
  partial = out^T.T @ W_proj_slice + bias   (per-head K=64 contractions)
"""

import sys
import types

for _p in ("/opt/trn_rl_repo",):
    if _p not in sys.path:
        sys.path.insert(0, _p)

import numpy as np
import ml_dtypes
from contextlib import ExitStack

# antenv.axon_hooks shim: lets run_bass_kernel_spmd find the NTFF profiling
# hook when BASS_TRACE=1 (the agent image's antenv lacks this module).
import antenv  # noqa: F401

if "antenv.axon_hooks" not in sys.modules:
    _m = types.ModuleType("antenv.axon_hooks")
    _m._hook = None

    def _set_hook(h):
        _m._hook = h

    def _get_hook():
        return _m._hook

    _m.set_axon_ntff_profile_hook = _set_hook
    _m.get_axon_ntff_profile_hook = _get_hook
    sys.modules["antenv.axon_hooks"] = _m
    try:
        from trn_agent_boot.trn_boot import _ntff_profile_via_ctypes

        hook = _ntff_profile_via_ctypes("/opt/axon/libaxon_pjrt.so")
        if hook is not None:
            _set_hook(hook)
    except Exception:
        pass

import concourse.bass as bass  # noqa: E402
import concourse.tile as tile  # noqa: E402
from concourse import mybir, bacc  # noqa: E402
from concourse import bass_utils  # noqa: E402

# No bucket storage in this container; artifacts stay local.
bass_utils.upload_artifacts = lambda tmpdir: f"local://{tmpdir}"

B, N, C = 2, 4096, 512
H, D = 8, 64
N_CORES = 8
SCALE = D ** -0.5

BF16 = mybir.dt.bfloat16
F32 = mybir.dt.float32
AF = mybir.ActivationFunctionType
BFNP = ml_dtypes.bfloat16

NI = N // 128   # 32 kidx / n chunks
NJ = N // 1024  # 4 q blocks
VW = 2 * (D + 1)  # 130: per-n-chunk vaug block (2 heads x (64 v + 1 ones))


def build_nc():
    nc = bacc.Bacc("TRN2", target_bir_lowering=False, debug=False)

    xt = nc.dram_tensor("xt", [4, 128, N], BF16, kind="ExternalInput").ap()
    wq = nc.dram_tensor("wq", [128, 512], BF16, kind="ExternalInput").ap()
    wk = nc.dram_tensor("wk", [128, 512], BF16, kind="ExternalInput").ap()
    wv = nc.dram_tensor("wv", [128, 512], BF16, kind="ExternalInput").ap()
    wp = nc.dram_tensor("wp", [64, 1024], BF16, kind="ExternalInput").ap()
    bias = nc.dram_tensor("bias", [1, 512], BF16, kind="ExternalInput").ap()
    out = nc.dram_tensor("out", [N, C], F32, kind="ExternalOutput").ap()

    with tile.TileContext(nc) as tc:
        with ExitStack() as ctx:
            const = ctx.enter_context(tc.tile_pool(name="const", bufs=1))
            sb = ctx.enter_context(tc.tile_pool(name="sb", bufs=1))
            expp = ctx.enter_context(tc.tile_pool(name="expp", bufs=4))
            invp = ctx.enter_context(tc.tile_pool(name="invp", bufs=2))
            outp = ctx.enter_context(tc.tile_pool(name="outp", bufs=3))

            twq = const.tile([128, 512], BF16)
            nc.sync.dma_start(twq[:], wq[:])
            twk = const.tile([128, 512], BF16)
            nc.sync.dma_start(twk[:], wk[:])
            twv = const.tile([128, 512], BF16)
            nc.sync.dma_start(twv[:], wv[:])
            twp = const.tile([64, 1024], BF16)
            nc.sync.dma_start(twp[:], wp[:])
            tbias = const.tile([1, 512], BF16)
            nc.sync.dma_start(tbias[:], bias[:])
            tones = const.tile([1, 128], BF16)
            nc.gpsimd.memset(tones[:], 1.0)

            qT = sb.tile([128, N], BF16)
            kT = sb.tile([128, N], BF16)
            vaug = sb.tile([128, NI * VW], BF16)
            nc.gpsimd.memset(vaug[:], 1.0)
            outT0 = sb.tile([64, N], BF16)
            outT1 = sb.tile([64, N], BF16)
            outTs = (outT0, outT1)

            # ---- stage A: QKV projections -------------------------------
            with tc.tile_pool(name="xtp", bufs=1) as xtp, tc.tile_pool(
                name="psA", bufs=2, space="PSUM"
            ) as psA:
                xts = []
                for k in range(4):
                    t = xtp.tile([128, N], BF16, tag=f"xt{k}")
                    nc.sync.dma_start(t[:], xt[k])
                    xts.append(t)
                for j8 in range(8):
                    s = bass.ts(j8, 512)
                    for w, dst in ((twq, qT), (twk, kT)):
                        ps = psA.tile([128, 512], F32, tag="qk")
                        for k in range(4):
                            nc.tensor.matmul(
                                ps[:],
                                w[:, bass.ts(k, 128)],
                                xts[k][:, s],
                                start=(k == 0),
                                stop=(k == 3),
                            )
                        nc.vector.tensor_copy(dst[:, s], ps[:])
                for jj in range(NI):
                    ps = psA.tile([128, 128], F32, tag="v")
                    for k in range(4):
                        nc.tensor.matmul(
                            ps[:],
                            xts[k][:, bass.ts(jj, 128)],
                            twv[:, bass.ts(k, 128)],
                            start=(k == 0),
                            stop=(k == 3),
                        )
                    dst = vaug[:, jj * VW : (jj + 1) * VW].rearrange(
                        "p (h c) -> p h c", h=2
                    )[:, :, 0:D]
                    src = ps[:].rearrange("p (h c) -> p h c", h=2)
                    nc.vector.tensor_copy(dst, src)

            # ---- stage B: scores^T -> exp -> AV (+den), normalize -------
            # ---- stage C: partial projection + bias ---------------------
            with tc.tile_pool(name="psS", bufs=2, space="PSUM") as psS, tc.tile_pool(
                name="psAV", bufs=2, space="PSUM"
            ) as psAV:
                for j in range(8):
                    qs = bass.ts(j, 512)
                    avs = [
                        psAV.tile([65, 512], F32, tag=f"av{t}", name=f"av{t}")
                        for t in range(2)
                    ]  # one per head
                    for i in range(NI):
                        ks = bass.ts(i, 128)
                        pss = psS.tile([128, 1024], F32, tag="s")
                        nc.tensor.matmul(
                            pss[:, 0:512],
                            kT[0:64, ks],
                            qT[0:64, qs],
                            start=True,
                            stop=True,
                        )
                        nc.tensor.matmul(
                            pss[:, 512:1024],
                            kT[64:128, ks],
                            qT[64:128, qs],
                            start=True,
                            stop=True,
                        )
                        est = expp.tile([128, 1024], BF16, tag="e")
                        nc.scalar.activation(est[:], pss[:], AF.Exp, scale=SCALE)
                        for h in range(2):
                            va = vaug[:, i * VW + h * 65 : i * VW + (h + 1) * 65]
                            nc.tensor.matmul(
                                avs[h][:],
                                va,
                                est[:, bass.ts(h, 512)],
                                start=(i == 0),
                                stop=(i == NI - 1),
                            )
                    for h in range(2):
                        inv = invp.tile([1, 512], BF16, tag="inv")
                        with nc.allow_low_precision(reason="softmax 1/den in bf16"):
                            nc.vector.reciprocal(inv[:], avs[h][64:65, :])
                        psb = psS.tile([64, 512], F32, tag="s")
                        nc.tensor.matmul(
                            psb[:], tones[0:1, 0:64], inv[:], start=True, stop=True
                        )
                        sbb = invp.tile([64, 512], BF16, tag="sbb")
                        nc.vector.tensor_copy(sbb[:], psb[:])
                        nc.vector.tensor_mul(
                            outTs[h][:, qs], avs[h][0:64, :], sbb[:]
                        )
                    for jj in range(j * 4, (j + 1) * 4):
                        s = bass.ts(jj, 128)
                        pp = psS.tile([128, 512], F32, tag="s")
                        nc.tensor.matmul(
                            pp[:], outT0[:, s], twp[:, 0:512], start=True, stop=False
                        )
                        nc.tensor.matmul(
                            pp[:], outT1[:, s], twp[:, 512:1024], start=False, stop=False
                        )
                        nc.tensor.matmul(
                            pp[:], tones[0:1, :], tbias[:], start=False, stop=True
                        )
                        ot = outp.tile([128, 512], F32, tag="o")
                        nc.vector.tensor_copy(ot[:], pp[:])
                        nc.sync.dma_start(out[s, :], ot[:])

    nc.compile()
    return nc


def _pack_w(wslice):
    # [512, 128] -> SBUF image [128, 4*128] with C-chunk k at cols k*128..
    return np.ascontiguousarray(
        wslice.reshape(4, 128, 128).transpose(1, 0, 2).reshape(128, 512)
    ).astype(BFNP)


_NC_CACHE = None
LAST_RESULT = None


def kernel(x, W_qkv, W_proj, b_proj):
    global _NC_CACHE, LAST_RESULT
    x = np.asarray(x, dtype=np.float32)
    W_qkv = np.asarray(W_qkv, dtype=np.float32)
    W_proj = np.asarray(W_proj, dtype=np.float32)
    b_proj = np.asarray(b_proj, dtype=np.float32)

    if _NC_CACHE is None:
        _NC_CACHE = build_nc()
    nc = _NC_CACHE

    in_maps = []
    for c in range(N_CORES):
        b = c // 4
        h0 = 2 * (c % 4)
        xtb = np.ascontiguousarray(x[b].T).reshape(4, 128, N).astype(BFNP)
        wq = _pack_w(W_qkv[:, h0 * 64 : h0 * 64 + 128])
        wk = _pack_w(W_qkv[:, 512 + h0 * 64 : 512 + h0 * 64 + 128])
        wv = _pack_w(W_qkv[:, 1024 + h0 * 64 : 1024 + h0 * 64 + 128])
        wp = np.ascontiguousarray(
            np.concatenate(
                [
                    W_proj[h0 * 64 : (h0 + 1) * 64, :],
                    W_proj[(h0 + 1) * 64 : (h0 + 2) * 64, :],
                ],
                axis=1,
            )
        ).astype(BFNP)
        bias = (
            b_proj[None, :].astype(BFNP)
            if c % 4 == 0
            else np.zeros((1, 512), dtype=BFNP)
        )
        in_maps.append(
            {"xt": xtb, "wq": wq, "wk": wk, "wv": wv, "wp": wp, "bias": bias}
        )

    res = bass_utils.run_bass_kernel_spmd(
        nc, in_maps, core_ids=list(range(N_CORES))
    )
    LAST_RESULT = res

    out = np.zeros((B, N, C), dtype=np.float32)
    for c in range(N_CORES):
        out[c // 4] += res.results[c]["out"]
    return out


# revision 12
# speedup vs baseline: 1.1802x; 1.0630x over previous
"""Multi-head attention (B=2, N=4096, C=512, H=8, d=64) on 8 Trainium2 NeuronCores.

Sharding: core c handles batch b = c//4 and heads {2*(c%4), 2*(c%4)+1}.
Each core computes its 2 heads' attention plus a partial output projection
(contraction over its 128 rows of W_proj); the host gather sums the 4
partials per batch (bias is added on the p==0 core of each batch).

On-device dataflow (transposed-scores formulation, no on-chip transposes):
  qT/kT [128=2*64 d-dims, 4096]  = W.T @ x.T      (x.T supplied by host)
  v_aug [128 n-chunk, 32*(65+65)] = x @ Wv with a ones column per head
  S^T[kidx, q] = kT.T_chunk @ qT  (two heads ride row-groups 0-1 / 2-3
                                   of the PE array concurrently, K=64 each)
  E = exp(S^T / 8)                (ScalarE, scale folded into the LUT affine)
  [out_unnorm^T; den] = v_aug.T @ E   (ones column makes row 64 the softmax
                                       denominator -- no extra pass)
  out^T = out_unnorm^T * (1/den)  (reciprocal + K=1 broadcast matmul)
  partial = out^T.T @ W_proj_slice + bias   (per-head K=64 contractions)
"""

import sys
import types

for _p in ("/opt/trn_rl_repo",):
    if _p not in sys.path:
        sys.path.insert(0, _p)

import numpy as np
import ml_dtypes
from contextlib import ExitStack

# antenv.axon_hooks shim: lets run_bass_kernel_spmd find the NTFF profiling
# hook when BASS_TRACE=1 (the agent image's antenv lacks this module).
import antenv  # noqa: F401

if "antenv.axon_hooks" not in sys.modules:
    _m = types.ModuleType("antenv.axon_hooks")
    _m._hook = None

    def _set_hook(h):
        _m._hook = h

    def _get_hook():
        return _m._hook

    _m.set_axon_ntff_profile_hook = _set_hook
    _m.get_axon_ntff_profile_hook = _get_hook
    sys.modules["antenv.axon_hooks"] = _m
    try:
        from trn_agent_boot.trn_boot import _ntff_profile_via_ctypes

        hook = _ntff_profile_via_ctypes("/opt/axon/libaxon_pjrt.so")
        if hook is not None:
            _set_hook(hook)
    except Exception:
        pass

import concourse.bass as bass  # noqa: E402
import concourse.tile as tile  # noqa: E402
from concourse import mybir, bacc  # noqa: E402
from concourse import bass_utils  # noqa: E402

# No bucket storage in this container; artifacts stay local.
bass_utils.upload_artifacts = lambda tmpdir: f"local://{tmpdir}"

B, N, C = 2, 4096, 512
H, D = 8, 64
N_CORES = 8
SCALE = D ** -0.5

BF16 = mybir.dt.bfloat16
F32 = mybir.dt.float32
AF = mybir.ActivationFunctionType
BFNP = ml_dtypes.bfloat16

NI = N // 128   # 32 kidx / n chunks
NJ = N // 1024  # 4 q blocks
VW = 2 * (D + 1)  # 130: per-n-chunk vaug block (2 heads x (64 v + 1 ones))


def build_nc():
    nc = bacc.Bacc("TRN2", target_bir_lowering=False, debug=False)

    xt = nc.dram_tensor("xt", [4, 128, N], BF16, kind="ExternalInput").ap()
    wq = nc.dram_tensor("wq", [128, 512], BF16, kind="ExternalInput").ap()
    wk = nc.dram_tensor("wk", [128, 512], BF16, kind="ExternalInput").ap()
    wv = nc.dram_tensor("wv", [128, 512], BF16, kind="ExternalInput").ap()
    wp = nc.dram_tensor("wp", [64, 1024], BF16, kind="ExternalInput").ap()
    bias = nc.dram_tensor("bias", [1, 512], BF16, kind="ExternalInput").ap()
    out = nc.dram_tensor("out", [N, C], F32, kind="ExternalOutput").ap()

    with tile.TileContext(nc) as tc:
        with ExitStack() as ctx:
            const = ctx.enter_context(tc.tile_pool(name="const", bufs=1))
            sb = ctx.enter_context(tc.tile_pool(name="sb", bufs=1))
            expp = ctx.enter_context(tc.tile_pool(name="expp", bufs=4))
            invp = ctx.enter_context(tc.tile_pool(name="invp", bufs=2))
            outp = ctx.enter_context(tc.tile_pool(name="outp", bufs=3))

            twq = const.tile([128, 512], BF16)
            nc.sync.dma_start(twq[:], wq[:])
            twk = const.tile([128, 512], BF16)
            nc.sync.dma_start(twk[:], wk[:])
            twv = const.tile([128, 512], BF16)
            nc.sync.dma_start(twv[:], wv[:])
            twp = const.tile([64, 1024], BF16)
            nc.sync.dma_start(twp[:], wp[:])
            tbias = const.tile([1, 512], BF16)
            nc.sync.dma_start(tbias[:], bias[:])
            tones = const.tile([1, 128], BF16)
            nc.gpsimd.memset(tones[:], 1.0)
            tones32 = const.tile([1, 64], F32)
            nc.gpsimd.memset(tones32[:], 1.0)

            qT = sb.tile([128, N], BF16)
            kT = sb.tile([128, N], BF16)
            vaug = sb.tile([128, NI * VW], BF16)
            nc.gpsimd.memset(vaug[:], 1.0)
            outT0 = sb.tile([64, N], BF16)
            outT1 = sb.tile([64, N], BF16)
            outTs = (outT0, outT1)

            xtp = ctx.enter_context(tc.tile_pool(name="xtp", bufs=1))
            psS = ctx.enter_context(tc.tile_pool(name="psS", bufs=2, space="PSUM"))
            psAV = ctx.enter_context(tc.tile_pool(name="psAV", bufs=1, space="PSUM"))
            psT = ctx.enter_context(tc.tile_pool(name="psT", bufs=2, space="PSUM"))

            # ---- stage A: QKV projections (psT shared with stage B tails
            # so stage B's score PSUM banks are independent of stage A) ----
            xts = []
            for k in range(4):
                t = xtp.tile([128, N], BF16, tag=f"xt{k}", name=f"xt{k}")
                nc.sync.dma_start(t[:], xt[k])
                xts.append(t)
            for j8 in range(8):
                s = bass.ts(j8, 512)
                for w, dst in ((twq, qT), (twk, kT)):
                    ps = psT.tile([128, 512], F32, tag="t", name="psqk")
                    for k in range(4):
                        nc.tensor.matmul(
                            ps[:],
                            w[:, bass.ts(k, 128)],
                            xts[k][:, s],
                            start=(k == 0),
                            stop=(k == 3),
                        )
                    nc.vector.tensor_copy(dst[:, s], ps[:])
                for jj in range(j8 * 4, (j8 + 1) * 4):
                    ps = psT.tile([128, 128], F32, tag="t", name="psv")
                    for k in range(4):
                        nc.tensor.matmul(
                            ps[:],
                            xts[k][:, bass.ts(jj, 128)],
                            twv[:, bass.ts(k, 128)],
                            start=(k == 0),
                            stop=(k == 3),
                        )
                    dst = vaug[:, jj * VW : (jj + 1) * VW].rearrange(
                        "p (h c) -> p h c", h=2
                    )[:, :, 0:D]
                    src = ps[:].rearrange("p (h c) -> p h c", h=2)
                    nc.vector.tensor_copy(dst, src)

            # ---- stage B: scores^T -> exp -> AV (+den), normalize -------
            # ---- stage C: partial projection + bias ---------------------
            if True:
                for j in range(8):
                    qs = bass.ts(j, 512)
                    avs = [
                        psAV.tile([65, 512], F32, tag=f"av{t}", name=f"av{t}")
                        for t in range(2)
                    ]  # one per head
                    for i in range(NI):
                        ks = bass.ts(i, 128)
                        pss = psS.tile([128, 1024], F32, tag="s")
                        nc.tensor.matmul(
                            pss[:, 0:512],
                            kT[0:64, ks],
                            qT[0:64, qs],
                            start=True,
                            stop=True,
                        )
                        nc.tensor.matmul(
                            pss[:, 512:1024],
                            kT[64:128, ks],
                            qT[64:128, qs],
                            start=True,
                            stop=True,
                        )
                        est = expp.tile([128, 1024], BF16, tag="e")
                        nc.scalar.activation(est[:], pss[:], AF.Exp, scale=SCALE)
                        for h in range(2):
                            va = vaug[:, i * VW + h * 65 : i * VW + (h + 1) * 65]
                            nc.tensor.matmul(
                                avs[h][:],
                                va,
                                est[:, bass.ts(h, 512)],
                                start=(i == 0),
                                stop=(i == NI - 1),
                            )
                    for h in range(2):
                        # evacuate PSUM accumulator fast so next j's AV can
                        # reuse the bank; the slow tail reads the SBUF copy
                        avsb = invp.tile([65, 512], F32, tag="avsb")
                        nc.vector.tensor_copy(avsb[:], avs[h][:])
                        inv = invp.tile([1, 512], BF16, tag="inv")
                        with nc.allow_low_precision(reason="softmax 1/den in bf16"):
                            nc.vector.reciprocal(inv[:], avsb[64:65, :])
                        psb = psT.tile([64, 512], F32, tag="t", name="psb")
                        nc.tensor.matmul(
                            psb[:], tones[0:1, 0:64], inv[:], start=True, stop=True
                        )
                        sbb = invp.tile([64, 512], BF16, tag="sbb")
                        nc.vector.tensor_copy(sbb[:], psb[:])
                        nc.vector.tensor_mul(
                            outTs[h][:, qs], avsb[0:64, :], sbb[:]
                        )
                    for jj in range(j * 4, (j + 1) * 4):
                        s = bass.ts(jj, 128)
                        pp = psT.tile([128, 512], F32, tag="t")
                        nc.tensor.matmul(
                            pp[:], outT0[:, s], twp[:, 0:512], start=True, stop=False
                        )
                        nc.tensor.matmul(
                            pp[:], outT1[:, s], twp[:, 512:1024], start=False, stop=False
                        )
                        nc.tensor.matmul(
                            pp[:], tones[0:1, :], tbias[:], start=False, stop=True
                        )
                        ot = outp.tile([128, 512], F32, tag="o")
                        nc.vector.tensor_copy(ot[:], pp[:])
                        nc.sync.dma_start(out[s, :], ot[:])

    nc.compile()
    return nc


def _pack_w(wslice):
    # [512, 128] -> SBUF image [128, 4*128] with C-chunk k at cols k*128..
    return np.ascontiguousarray(
        wslice.reshape(4, 128, 128).transpose(1, 0, 2).reshape(128, 512)
    ).astype(BFNP)


_NC_CACHE = None
LAST_RESULT = None


def kernel(x, W_qkv, W_proj, b_proj):
    global _NC_CACHE, LAST_RESULT
    x = np.asarray(x, dtype=np.float32)
    W_qkv = np.asarray(W_qkv, dtype=np.float32)
    W_proj = np.asarray(W_proj, dtype=np.float32)
    b_proj = np.asarray(b_proj, dtype=np.float32)

    if _NC_CACHE is None:
        _NC_CACHE = build_nc()
    nc = _NC_CACHE

    in_maps = []
    for c in range(N_CORES):
        b = c // 4
        h0 = 2 * (c % 4)
        xtb = np.ascontiguousarray(x[b].T).reshape(4, 128, N).astype(BFNP)
        wq = _pack_w(W_qkv[:, h0 * 64 : h0 * 64 + 128])
        wk = _pack_w(W_qkv[:, 512 + h0 * 64 : 512 + h0 * 64 + 128])
        wv = _pack_w(W_qkv[:, 1024 + h0 * 64 : 1024 + h0 * 64 + 128])
        wp = np.ascontiguousarray(
            np.concatenate(
                [
                    W_proj[h0 * 64 : (h0 + 1) * 64, :],
                    W_proj[(h0 + 1) * 64 : (h0 + 2) * 64, :],
                ],
                axis=1,
            )
        ).astype(BFNP)
        bias = (
            b_proj[None, :].astype(BFNP)
            if c % 4 == 0
            else np.zeros((1, 512), dtype=BFNP)
        )
        in_maps.append(
            {"xt": xtb, "wq": wq, "wk": wk, "wv": wv, "wp": wp, "bias": bias}
        )

    res = bass_utils.run_bass_kernel_spmd(
        nc, in_maps, core_ids=list(range(N_CORES))
    )
    LAST_RESULT = res

    out = np.zeros((B, N, C), dtype=np.float32)
    for c in range(N_CORES):
        out[c // 4] += res.results[c]["out"]
    return out


# revision 14
# speedup vs baseline: 1.3230x; 1.1210x over previous
"""Multi-head attention (B=2, N=4096, C=512, H=8, d=64) on 8 Trainium2 NeuronCores.

Sharding: core c handles batch b = c//4 and heads {2*(c%4), 2*(c%4)+1}.
Each core computes its 2 heads' attention plus a partial output projection
(contraction over its 128 rows of W_proj); the host gather sums the 4
partials per batch (bias is added on the p==0 core of each batch).

On-device dataflow (transposed-scores formulation, no on-chip transposes):
  qT/kT [128=2*64 d-dims, 4096]  = W.T @ x.T      (x.T supplied by host)
  v_aug [128 n-chunk, 32*(65+65)] = x @ Wv with a ones column per head
  S^T[kidx, q] = kT.T_chunk @ qT  (two heads ride row-groups 0-1 / 2-3
                                   of the PE array concurrently, K=64 each)
  E = exp(S^T / 8)                (ScalarE, scale folded into the LUT affine)
  [out_unnorm^T; den] = v_aug.T @ E   (ones column makes row 64 the softmax
                                       denominator -- no extra pass)
  out^T = out_unnorm^T * (1/den)  (reciprocal + K=1 broadcast matmul)
  partial = out^T.T @ W_proj_slice + bias   (per-head K=64 contractions)
"""

import sys
import types

for _p in ("/opt/trn_rl_repo",):
    if _p not in sys.path:
        sys.path.insert(0, _p)

import numpy as np
import ml_dtypes
from contextlib import ExitStack

# antenv.axon_hooks shim: lets run_bass_kernel_spmd find the NTFF profiling
# hook when BASS_TRACE=1 (the agent image's antenv lacks this module).
import antenv  # noqa: F401

if "antenv.axon_hooks" not in sys.modules:
    _m = types.ModuleType("antenv.axon_hooks")
    _m._hook = None

    def _set_hook(h):
        _m._hook = h

    def _get_hook():
        return _m._hook

    _m.set_axon_ntff_profile_hook = _set_hook
    _m.get_axon_ntff_profile_hook = _get_hook
    sys.modules["antenv.axon_hooks"] = _m
    try:
        from trn_agent_boot.trn_boot import _ntff_profile_via_ctypes

        hook = _ntff_profile_via_ctypes("/opt/axon/libaxon_pjrt.so")
        if hook is not None:
            _set_hook(hook)
    except Exception:
        pass

import concourse.bass as bass  # noqa: E402
import concourse.tile as tile  # noqa: E402
from concourse import mybir, bacc  # noqa: E402
from concourse import bass_utils  # noqa: E402

# No bucket storage in this container; artifacts stay local.
bass_utils.upload_artifacts = lambda tmpdir: f"local://{tmpdir}"

B, N, C = 2, 4096, 512
H, D = 8, 64
N_CORES = 8
SCALE = D ** -0.5

BF16 = mybir.dt.bfloat16
F32 = mybir.dt.float32
AF = mybir.ActivationFunctionType
BFNP = ml_dtypes.bfloat16

NI = N // 128   # 32 kidx / n chunks
NJ = N // 1024  # 4 q blocks
VW = 2 * (D + 1)  # 130: per-n-chunk vaug block (2 heads x (64 v + 1 ones))


def build_nc():
    nc = bacc.Bacc("TRN2", target_bir_lowering=False, debug=False)

    xt = nc.dram_tensor("xt", [4, 128, N], BF16, kind="ExternalInput").ap()
    wq = nc.dram_tensor("wq", [128, 512], BF16, kind="ExternalInput").ap()
    wk = nc.dram_tensor("wk", [128, 512], BF16, kind="ExternalInput").ap()
    wv = nc.dram_tensor("wv", [128, 512], BF16, kind="ExternalInput").ap()
    wp = nc.dram_tensor("wp", [64, 1024], BF16, kind="ExternalInput").ap()
    bias = nc.dram_tensor("bias", [1, 512], BF16, kind="ExternalInput").ap()
    out = nc.dram_tensor("out", [N, C], F32, kind="ExternalOutput").ap()

    with tile.TileContext(nc) as tc:
        with ExitStack() as ctx:
            const = ctx.enter_context(tc.tile_pool(name="const", bufs=1))
            sb = ctx.enter_context(tc.tile_pool(name="sb", bufs=1))
            expp = ctx.enter_context(tc.tile_pool(name="expp", bufs=4))
            invp = ctx.enter_context(tc.tile_pool(name="invp", bufs=2))
            outp = ctx.enter_context(tc.tile_pool(name="outp", bufs=3))

            twq = const.tile([128, 512], BF16)
            nc.sync.dma_start(twq[:], wq[:])
            twk = const.tile([128, 512], BF16)
            nc.sync.dma_start(twk[:], wk[:])
            twv = const.tile([128, 512], BF16)
            nc.sync.dma_start(twv[:], wv[:])
            twp = const.tile([64, 1024], BF16)
            nc.sync.dma_start(twp[:], wp[:])
            tbias = const.tile([1, 512], BF16)
            nc.sync.dma_start(tbias[:], bias[:])
            tones = const.tile([1, 128], BF16)
            nc.gpsimd.memset(tones[:], 1.0)
            tones32 = const.tile([1, 64], F32)
            nc.gpsimd.memset(tones32[:], 1.0)

            qT = sb.tile([128, N], BF16)
            kT = sb.tile([128, N], BF16)
            vaug = sb.tile([128, NI * VW], BF16)
            nc.gpsimd.memset(vaug[:], 1.0)
            outT0 = sb.tile([64, N], BF16)
            outT1 = sb.tile([64, N], BF16)
            outTs = (outT0, outT1)

            xtp = ctx.enter_context(tc.tile_pool(name="xtp", bufs=1))
            psS = ctx.enter_context(tc.tile_pool(name="psS", bufs=2, space="PSUM"))
            psAV = ctx.enter_context(tc.tile_pool(name="psAV", bufs=1, space="PSUM"))
            psT = ctx.enter_context(tc.tile_pool(name="psT", bufs=2, space="PSUM"))

            # ---- stage A: QKV projections (psT shared with stage B tails
            # so stage B's score PSUM banks are independent of stage A) ----
            xts = []
            for k in range(4):
                t = xtp.tile([128, N], BF16, tag=f"xt{k}", name=f"xt{k}")
                xts.append(t)
            for col in range(2):
                for k in range(4):
                    cs = bass.ts(col, N // 2)
                    nc.sync.dma_start(xts[k][:, cs], xt[k][:, cs])
            for j8 in range(8):
                s = bass.ts(j8, 512)
                for w, dst in ((twq, qT), (twk, kT)):
                    ps = psT.tile([128, 512], F32, tag="t", name="psqk")
                    for k in range(4):
                        nc.tensor.matmul(
                            ps[:],
                            w[:, bass.ts(k, 128)],
                            xts[k][:, s],
                            start=(k == 0),
                            stop=(k == 3),
                        )
                    nc.vector.tensor_copy(dst[:, s], ps[:])
                for jj in range(j8 * 4, (j8 + 1) * 4):
                    ps = psT.tile([128, 128], F32, tag="t", name="psv")
                    for k in range(4):
                        nc.tensor.matmul(
                            ps[:],
                            xts[k][:, bass.ts(jj, 128)],
                            twv[:, bass.ts(k, 128)],
                            start=(k == 0),
                            stop=(k == 3),
                        )
                    dst = vaug[:, jj * VW : (jj + 1) * VW].rearrange(
                        "p (h c) -> p h c", h=2
                    )[:, :, 0:D]
                    src = ps[:].rearrange("p (h c) -> p h c", h=2)
                    nc.vector.tensor_copy(dst, src)

            # ---- stage B: scores^T -> exp -> AV (+den), normalize -------
            # ---- stage C: partial projection + bias ---------------------
            # Tails (normalize + projection of block j) are emitted in the
            # middle of block j+1's i-loop: the PE queue is strict FIFO, so
            # matmuls that wait on the slow DVE reciprocal chain must sit
            # behind enough independent PE work to never stall the queue.
            def emit_bcast(st):
                h = st["h"]
                psb = psT.tile([64, 512], F32, tag="t", name="psb")
                nc.tensor.matmul(
                    psb[:], tones[0:1, 0:64], st["inv"][:], start=True, stop=True
                )
                sbb = invp.tile([64, 512], BF16, tag="sbb", name="sbb")
                nc.vector.tensor_copy(sbb[:], psb[:])
                nc.vector.tensor_mul(
                    outTs[h][:, st["qs"]], st["avsb"][0:64, :], sbb[:]
                )

            def emit_proj(j, k):
                jj = j * 4 + k
                s = bass.ts(jj, 128)
                pp = psT.tile([128, 512], F32, tag="t", name="pp")
                nc.tensor.matmul(
                    pp[:], outT0[:, s], twp[:, 0:512], start=True, stop=False
                )
                nc.tensor.matmul(
                    pp[:], outT1[:, s], twp[:, 512:1024], start=False, stop=False
                )
                nc.tensor.matmul(
                    pp[:], tones[0:1, :], tbias[:], start=False, stop=True
                )
                ot = outp.tile([128, 512], F32, tag="o", name="ot")
                nc.vector.tensor_copy(ot[:], pp[:])
                nc.sync.dma_start(out[s, :], ot[:])

            prev = None  # pending tail of block j-1
            for j in range(8):
                qs = bass.ts(j, 512)
                avs = [
                    psAV.tile([65, 512], F32, tag=f"av{t}", name=f"av{t}")
                    for t in range(2)
                ]  # one per head
                for i in range(NI):
                    if prev is not None:
                        if i == 6:
                            emit_bcast(prev["n"][0])
                        elif i == 10:
                            emit_bcast(prev["n"][1])
                        elif i >= 16 and i % 4 == 0:  # 16, 20, 24, 28
                            emit_proj(prev["j"], (i - 16) // 4)
                    ks = bass.ts(i, 128)
                    pss = psS.tile([128, 1024], F32, tag="s")
                    nc.tensor.matmul(
                        pss[:, 0:512], kT[0:64, ks], qT[0:64, qs],
                        start=True, stop=True,
                    )
                    nc.tensor.matmul(
                        pss[:, 512:1024], kT[64:128, ks], qT[64:128, qs],
                        start=True, stop=True,
                    )
                    est = expp.tile([128, 1024], BF16, tag="e")
                    nc.scalar.activation(est[:], pss[:], AF.Exp, scale=SCALE)
                    for h in range(2):
                        va = vaug[:, i * VW + h * 65 : i * VW + (h + 1) * 65]
                        nc.tensor.matmul(
                            avs[h][:], va, est[:, bass.ts(h, 512)],
                            start=(i == 0), stop=(i == NI - 1),
                        )
                # evacuate PSUM accumulators fast (releases the avs banks
                # for j+1) and start the reciprocals; the rest of the tail
                # is emitted inside block j+1's i-loop.
                norms = []
                for h in range(2):
                    avsb = invp.tile([65, 512], F32, tag="avsb", name="avsb")
                    nc.vector.tensor_copy(avsb[:], avs[h][:])
                    inv = invp.tile([1, 512], BF16, tag="inv", name="inv")
                    with nc.allow_low_precision(reason="softmax 1/den in bf16"):
                        nc.vector.reciprocal(inv[:], avsb[64:65, :])
                    norms.append({"h": h, "qs": qs, "avsb": avsb, "inv": inv})
                prev = {"j": j, "n": norms}
            # final block's tail
            for st in prev["n"]:
                emit_bcast(st)
            for k in range(4):
                emit_proj(prev["j"], k)

    nc.compile()
    return nc


def _pack_w(wslice):
    # [512, 128] -> SBUF image [128, 4*128] with C-chunk k at cols k*128..
    return np.ascontiguousarray(
        wslice.reshape(4, 128, 128).transpose(1, 0, 2).reshape(128, 512)
    ).astype(BFNP)


_NC_CACHE = None
LAST_RESULT = None


def kernel(x, W_qkv, W_proj, b_proj):
    global _NC_CACHE, LAST_RESULT
    x = np.asarray(x, dtype=np.float32)
    W_qkv = np.asarray(W_qkv, dtype=np.float32)
    W_proj = np.asarray(W_proj, dtype=np.float32)
    b_proj = np.asarray(b_proj, dtype=np.float32)

    if _NC_CACHE is None:
        _NC_CACHE = build_nc()
    nc = _NC_CACHE

    in_maps = []
    for c in range(N_CORES):
        b = c // 4
        h0 = 2 * (c % 4)
        xtb = np.ascontiguousarray(x[b].T).reshape(4, 128, N).astype(BFNP)
        wq = _pack_w(W_qkv[:, h0 * 64 : h0 * 64 + 128])
        wk = _pack_w(W_qkv[:, 512 + h0 * 64 : 512 + h0 * 64 + 128])
        wv = _pack_w(W_qkv[:, 1024 + h0 * 64 : 1024 + h0 * 64 + 128])
        wp = np.ascontiguousarray(
            np.concatenate(
                [
                    W_proj[h0 * 64 : (h0 + 1) * 64, :],
                    W_proj[(h0 + 1) * 64 : (h0 + 2) * 64, :],
                ],
                axis=1,
            )
        ).astype(BFNP)
        bias = (
            b_proj[None, :].astype(BFNP)
            if c % 4 == 0
            else np.zeros((1, 512), dtype=BFNP)
        )
        in_maps.append(
            {"xt": xtb, "wq": wq, "wk": wk, "wv": wv, "wp": wp, "bias": bias}
        )

    res = bass_utils.run_bass_kernel_spmd(
        nc, in_maps, core_ids=list(range(N_CORES))
    )
    LAST_RESULT = res

    out = np.zeros((B, N, C), dtype=np.float32)
    for c in range(N_CORES):
        out[c // 4] += res.results[c]["out"]
    return out


# revision 16
# speedup vs baseline: 1.3603x; 1.0282x over previous
"""Multi-head attention (B=2, N=4096, C=512, H=8, d=64) on 8 Trainium2 NeuronCores.

Sharding: core c handles batch b = c//4 and heads {2*(c%4), 2*(c%4)+1}.
Each core computes its 2 heads' attention plus a partial output projection
(contraction over its 128 rows of W_proj); the host gather sums the 4
partials per batch (bias is added on the p==0 core of each batch).

On-device dataflow (transposed-scores formulation, no on-chip transposes):
  qT/kT [128=2*64 d-dims, 4096]  = W.T @ x.T      (x.T supplied by host)
  v_aug [128 n-chunk, 32*(65+65)] = x @ Wv with a ones column per head
  S^T[kidx, q] = kT.T_chunk @ qT  (two heads ride row-groups 0-1 / 2-3
                                   of the PE array concurrently, K=64 each)
  E = exp(S^T / 8)                (ScalarE, scale folded into the LUT affine)
  [out_unnorm^T; den] = v_aug.T @ E   (ones column makes row 64 the softmax
                                       denominator -- no extra pass)
  out^T = out_unnorm^T * (1/den)  (reciprocal + K=1 broadcast matmul)
  partial = out^T.T @ W_proj_slice + bias   (per-head K=64 contractions)
"""

import sys
import types

for _p in ("/opt/trn_rl_repo",):
    if _p not in sys.path:
        sys.path.insert(0, _p)

import numpy as np
import ml_dtypes
from contextlib import ExitStack

# antenv.axon_hooks shim: lets run_bass_kernel_spmd find the NTFF profiling
# hook when BASS_TRACE=1 (the agent image's antenv lacks this module).
import antenv  # noqa: F401

if "antenv.axon_hooks" not in sys.modules:
    _m = types.ModuleType("antenv.axon_hooks")
    _m._hook = None

    def _set_hook(h):
        _m._hook = h

    def _get_hook():
        return _m._hook

    _m.set_axon_ntff_profile_hook = _set_hook
    _m.get_axon_ntff_profile_hook = _get_hook
    sys.modules["antenv.axon_hooks"] = _m
    try:
        from trn_agent_boot.trn_boot import _ntff_profile_via_ctypes

        hook = _ntff_profile_via_ctypes("/opt/axon/libaxon_pjrt.so")
        if hook is not None:
            _set_hook(hook)
    except Exception:
        pass

import concourse.bass as bass  # noqa: E402
import concourse.tile as tile  # noqa: E402
from concourse import mybir, bacc  # noqa: E402
from concourse import bass_utils  # noqa: E402

# No bucket storage in this container; artifacts stay local.
bass_utils.upload_artifacts = lambda tmpdir: f"local://{tmpdir}"

B, N, C = 2, 4096, 512
H, D = 8, 64
N_CORES = 8
SCALE = D ** -0.5

BF16 = mybir.dt.bfloat16
F32 = mybir.dt.float32
AF = mybir.ActivationFunctionType
BFNP = ml_dtypes.bfloat16

NI = N // 128   # 32 kidx / n chunks
NJ = N // 1024  # 4 q blocks
VW = 2 * (D + 1)  # 130: per-n-chunk vaug block (2 heads x (64 v + 1 ones))


def build_nc():
    nc = bacc.Bacc("TRN2", target_bir_lowering=False, debug=False)

    xt = nc.dram_tensor("xt", [4, 128, N], BF16, kind="ExternalInput").ap()
    wq = nc.dram_tensor("wq", [128, 512], BF16, kind="ExternalInput").ap()
    wk = nc.dram_tensor("wk", [128, 512], BF16, kind="ExternalInput").ap()
    wv = nc.dram_tensor("wv", [128, 512], BF16, kind="ExternalInput").ap()
    wp = nc.dram_tensor("wp", [64, 1024], BF16, kind="ExternalInput").ap()
    bias = nc.dram_tensor("bias", [1, 512], BF16, kind="ExternalInput").ap()
    out = nc.dram_tensor("out", [N, C], F32, kind="ExternalOutput").ap()

    with tile.TileContext(nc) as tc:
        with ExitStack() as ctx:
            const = ctx.enter_context(tc.tile_pool(name="const", bufs=1))
            sb = ctx.enter_context(tc.tile_pool(name="sb", bufs=1))
            expp = ctx.enter_context(tc.tile_pool(name="expp", bufs=4))
            invp = ctx.enter_context(tc.tile_pool(name="invp", bufs=2))
            outp = ctx.enter_context(tc.tile_pool(name="outp", bufs=3))

            twq = const.tile([128, 512], BF16)
            nc.sync.dma_start(twq[:], wq[:])
            twk = const.tile([128, 512], BF16)
            nc.sync.dma_start(twk[:], wk[:])
            twv = const.tile([128, 512], BF16)
            nc.sync.dma_start(twv[:], wv[:])
            twp = const.tile([64, 1024], BF16)
            nc.sync.dma_start(twp[:], wp[:])
            tbias = const.tile([1, 512], BF16)
            nc.sync.dma_start(tbias[:], bias[:])
            tones = const.tile([1, 128], BF16)
            nc.gpsimd.memset(tones[:], 1.0)
            tones32 = const.tile([1, 64], F32)
            nc.gpsimd.memset(tones32[:], 1.0)

            qT = sb.tile([128, N], BF16)
            kT = sb.tile([128, N], BF16)
            vaug = sb.tile([128, NI * VW], BF16)
            nc.gpsimd.memset(vaug[:], 1.0)
            outT0 = sb.tile([64, N], BF16)
            outT1 = sb.tile([64, N], BF16)
            outTs = (outT0, outT1)

            xtp = ctx.enter_context(tc.tile_pool(name="xtp", bufs=1))
            psS = ctx.enter_context(tc.tile_pool(name="psS", bufs=2, space="PSUM"))
            psAV = ctx.enter_context(tc.tile_pool(name="psAV", bufs=1, space="PSUM"))
            psT = ctx.enter_context(tc.tile_pool(name="psT", bufs=2, space="PSUM"))

            # ---- stage A: QKV projections ------------------------------
            # Emitted as deadline-scheduled tasks threaded into the first two
            # j-blocks' i-loops (the PE queue is strict FIFO; anything emitted
            # before the first score matmul delays the first exp).
            xts = []
            for k in range(4):
                t = xtp.tile([128, N], BF16, tag=f"xt{k}", name=f"xt{k}")
                xts.append(t)
            for col in range(2):
                for k in range(4):
                    cs = bass.ts(col, N // 2)
                    nc.sync.dma_start(xts[k][:, cs], xt[k][:, cs])

            def emit_qk(j8, which):
                s_ = bass.ts(j8, 512)
                w, dst = (twq, qT) if which == "q" else (twk, kT)
                ps = psT.tile([128, 512], F32, tag="t", name="psqk")
                for k in range(4):
                    nc.tensor.matmul(
                        ps[:], w[:, bass.ts(k, 128)], xts[k][:, s_],
                        start=(k == 0), stop=(k == 3),
                    )
                nc.vector.tensor_copy(dst[:, s_], ps[:])

            def emit_v(jj):
                ps = psT.tile([128, 128], F32, tag="t", name="psv")
                for k in range(4):
                    nc.tensor.matmul(
                        ps[:], xts[k][:, bass.ts(jj, 128)], twv[:, bass.ts(k, 128)],
                        start=(k == 0), stop=(k == 3),
                    )
                dst = vaug[:, jj * VW : (jj + 1) * VW].rearrange(
                    "p (h c) -> p h c", h=2
                )[:, :, 0:D]
                src = ps[:].rearrange("p (h c) -> p h c", h=2)
                nc.vector.tensor_copy(dst, src)

            # (deadline in global i-steps, emitter) — qk k-chunk c feeds
            # scores at step 4c; v chunk jj feeds the AV matmul at step jj;
            # qk q-chunk j8 feeds block j8 (step 32*j8).
            stage_a_tasks = []
            for c in range(1, 8):
                stage_a_tasks.append((4 * c - 4, lambda c=c: emit_qk(c, "k")))
            for jj in range(4, NI):
                stage_a_tasks.append((jj - 2, lambda jj=jj: emit_v(jj)))
            for j8 in range(1, 8):
                stage_a_tasks.append((32 * j8 - 6, lambda j8=j8: emit_qk(j8, "q")))
            stage_a_tasks.sort(key=lambda t: t[0])
            stage_a_tasks = list(stage_a_tasks)

            # prologue: what step 0 needs
            emit_qk(0, "q")
            emit_qk(0, "k")
            for jj in range(4):
                emit_v(jj)

            # ---- stage B: scores^T -> exp -> AV (+den), normalize -------
            # ---- stage C: partial projection + bias ---------------------
            # Tails (normalize + projection of block j) are emitted in the
            # middle of block j+1's i-loop: the PE queue is strict FIFO, so
            # matmuls that wait on the slow DVE reciprocal chain must sit
            # behind enough independent PE work to never stall the queue.
            def emit_bcast(st):
                h = st["h"]
                psb = psT.tile([64, 512], F32, tag="t", name="psb")
                nc.tensor.matmul(
                    psb[:], tones[0:1, 0:64], st["inv"][:], start=True, stop=True
                )
                sbb = invp.tile([64, 512], BF16, tag="sbb", name="sbb")
                nc.vector.tensor_copy(sbb[:], psb[:])
                nc.vector.tensor_mul(
                    outTs[h][:, st["qs"]], st["avsb"][0:64, :], sbb[:]
                )

            def emit_proj(j, k):
                jj = j * 4 + k
                s = bass.ts(jj, 128)
                pp = psT.tile([128, 512], F32, tag="t", name="pp")
                nc.tensor.matmul(
                    pp[:], outT0[:, s], twp[:, 0:512], start=True, stop=False
                )
                nc.tensor.matmul(
                    pp[:], outT1[:, s], twp[:, 512:1024], start=False, stop=False
                )
                nc.tensor.matmul(
                    pp[:], tones[0:1, :], tbias[:], start=False, stop=True
                )
                ot = outp.tile([128, 512], F32, tag="o", name="ot")
                nc.vector.tensor_copy(ot[:], pp[:])
                nc.sync.dma_start(out[s, :], ot[:])

            prev = None  # pending tail of block j-1
            for j in range(8):
                qs = bass.ts(j, 512)
                avs = [
                    psAV.tile([65, 512], F32, tag=f"av{t}", name=f"av{t}")
                    for t in range(2)
                ]  # one per head
                for i in range(NI):
                    gs = j * NI + i
                    while stage_a_tasks and stage_a_tasks[0][0] <= gs + 3:
                        stage_a_tasks.pop(0)[1]()
                    if prev is not None:
                        if i == 6:
                            emit_bcast(prev["n"][0])
                        elif i == 10:
                            emit_bcast(prev["n"][1])
                        elif i >= 16 and i % 4 == 0:  # 16, 20, 24, 28
                            emit_proj(prev["j"], (i - 16) // 4)
                    ks = bass.ts(i, 128)
                    pss = psS.tile([128, 1024], F32, tag="s")
                    nc.tensor.matmul(
                        pss[:, 0:512], kT[0:64, ks], qT[0:64, qs],
                        start=True, stop=True,
                    )
                    nc.tensor.matmul(
                        pss[:, 512:1024], kT[64:128, ks], qT[64:128, qs],
                        start=True, stop=True,
                    )
                    est = expp.tile([128, 1024], BF16, tag="e")
                    nc.scalar.activation(est[:], pss[:], AF.Exp, scale=SCALE)
                    for h in range(2):
                        va = vaug[:, i * VW + h * 65 : i * VW + (h + 1) * 65]
                        nc.tensor.matmul(
                            avs[h][:], va, est[:, bass.ts(h, 512)],
                            start=(i == 0), stop=(i == NI - 1),
                        )
                # evacuate PSUM accumulators fast (releases the avs banks
                # for j+1) and start the reciprocals; the rest of the tail
                # is emitted inside block j+1's i-loop.
                norms = []
                for h in range(2):
                    avsb = invp.tile([65, 512], F32, tag="avsb", name="avsb")
                    nc.vector.tensor_copy(avsb[:], avs[h][:])
                    inv = invp.tile([1, 512], BF16, tag="inv", name="inv")
                    with nc.allow_low_precision(reason="softmax 1/den in bf16"):
                        nc.vector.reciprocal(inv[:], avsb[64:65, :])
                    norms.append({"h": h, "qs": qs, "avsb": avsb, "inv": inv})
                prev = {"j": j, "n": norms}
            # final block's tail
            for st in prev["n"]:
                emit_bcast(st)
            for k in range(4):
                emit_proj(prev["j"], k)

    nc.compile()
    return nc


def _pack_w(wslice):
    # [512, 128] -> SBUF image [128, 4*128] with C-chunk k at cols k*128..
    return np.ascontiguousarray(
        wslice.reshape(4, 128, 128).transpose(1, 0, 2).reshape(128, 512)
    ).astype(BFNP)


_NC_CACHE = None
LAST_RESULT = None


def kernel(x, W_qkv, W_proj, b_proj):
    global _NC_CACHE, LAST_RESULT
    x = np.asarray(x, dtype=np.float32)
    W_qkv = np.asarray(W_qkv, dtype=np.float32)
    W_proj = np.asarray(W_proj, dtype=np.float32)
    b_proj = np.asarray(b_proj, dtype=np.float32)

    if _NC_CACHE is None:
        _NC_CACHE = build_nc()
    nc = _NC_CACHE

    in_maps = []
    for c in range(N_CORES):
        b = c // 4
        h0 = 2 * (c % 4)
        xtb = np.ascontiguousarray(x[b].T).reshape(4, 128, N).astype(BFNP)
        wq = _pack_w(W_qkv[:, h0 * 64 : h0 * 64 + 128])
        wk = _pack_w(W_qkv[:, 512 + h0 * 64 : 512 + h0 * 64 + 128])
        wv = _pack_w(W_qkv[:, 1024 + h0 * 64 : 1024 + h0 * 64 + 128])
        wp = np.ascontiguousarray(
            np.concatenate(
                [
                    W_proj[h0 * 64 : (h0 + 1) * 64, :],
                    W_proj[(h0 + 1) * 64 : (h0 + 2) * 64, :],
                ],
                axis=1,
            )
        ).astype(BFNP)
        bias = (
            b_proj[None, :].astype(BFNP)
            if c % 4 == 0
            else np.zeros((1, 512), dtype=BFNP)
        )
        in_maps.append(
            {"xt": xtb, "wq": wq, "wk": wk, "wv": wv, "wp": wp, "bias": bias}
        )

    res = bass_utils.run_bass_kernel_spmd(
        nc, in_maps, core_ids=list(range(N_CORES))
    )
    LAST_RESULT = res

    out = np.zeros((B, N, C), dtype=np.float32)
    for c in range(N_CORES):
        out[c // 4] += res.results[c]["out"]
    return out


# revision 18
# speedup vs baseline: 1.3820x; 1.0160x over previous
"""Multi-head attention (B=2, N=4096, C=512, H=8, d=64) on 8 Trainium2 NeuronCores.

Sharding: core c handles batch b = c//4 and heads {2*(c%4), 2*(c%4)+1}.
Each core computes its 2 heads' attention plus a partial output projection
(contraction over its 128 rows of W_proj); the host gather sums the 4
partials per batch (bias is added on the p==0 core of each batch).

On-device dataflow (transposed-scores formulation, no on-chip transposes):
  qT/kT [128=2*64 d-dims, 4096]  = W.T @ x.T      (x.T supplied by host)
  v_aug [128 n-chunk, 32*(65+65)] = x @ Wv with a ones column per head
  S^T[kidx, q] = kT.T_chunk @ qT  (two heads ride row-groups 0-1 / 2-3
                                   of the PE array concurrently, K=64 each)
  E = exp(S^T / 8)                (ScalarE, scale folded into the LUT affine)
  [out_unnorm^T; den] = v_aug.T @ E   (ones column makes row 64 the softmax
                                       denominator -- no extra pass)
  out^T = out_unnorm^T * (1/den)  (reciprocal + K=1 broadcast matmul)
  partial = out^T.T @ W_proj_slice + bias   (per-head K=64 contractions)
"""

import sys
import types

for _p in ("/opt/trn_rl_repo",):
    if _p not in sys.path:
        sys.path.insert(0, _p)

import numpy as np
import ml_dtypes
from contextlib import ExitStack

# antenv.axon_hooks shim: lets run_bass_kernel_spmd find the NTFF profiling
# hook when BASS_TRACE=1 (the agent image's antenv lacks this module).
import antenv  # noqa: F401

if "antenv.axon_hooks" not in sys.modules:
    _m = types.ModuleType("antenv.axon_hooks")
    _m._hook = None

    def _set_hook(h):
        _m._hook = h

    def _get_hook():
        return _m._hook

    _m.set_axon_ntff_profile_hook = _set_hook
    _m.get_axon_ntff_profile_hook = _get_hook
    sys.modules["antenv.axon_hooks"] = _m
    try:
        from trn_agent_boot.trn_boot import _ntff_profile_via_ctypes

        hook = _ntff_profile_via_ctypes("/opt/axon/libaxon_pjrt.so")
        if hook is not None:
            _set_hook(hook)
    except Exception:
        pass

import concourse.bass as bass  # noqa: E402
import concourse.tile as tile  # noqa: E402
from concourse import mybir, bacc  # noqa: E402
from concourse import bass_utils  # noqa: E402

# No bucket storage in this container; artifacts stay local.
bass_utils.upload_artifacts = lambda tmpdir: f"local://{tmpdir}"

B, N, C = 2, 4096, 512
H, D = 8, 64
N_CORES = 8
SCALE = D ** -0.5

BF16 = mybir.dt.bfloat16
F32 = mybir.dt.float32
AF = mybir.ActivationFunctionType
BFNP = ml_dtypes.bfloat16

NI = N // 128   # 32 kidx / n chunks
NJ = N // 1024  # 4 q blocks
VW = 2 * (D + 1)  # 130: per-n-chunk vaug block (2 heads x (64 v + 1 ones))


def build_nc():
    nc = bacc.Bacc("TRN2", target_bir_lowering=False, debug=False)

    xt = nc.dram_tensor("xt", [4, 128, N], BF16, kind="ExternalInput").ap()
    wq = nc.dram_tensor("wq", [128, 512], BF16, kind="ExternalInput").ap()
    wk = nc.dram_tensor("wk", [128, 512], BF16, kind="ExternalInput").ap()
    wv = nc.dram_tensor("wv", [128, 512], BF16, kind="ExternalInput").ap()
    wp = nc.dram_tensor("wp", [64, 1024], BF16, kind="ExternalInput").ap()
    bias = nc.dram_tensor("bias", [1, 512], BF16, kind="ExternalInput").ap()
    out = nc.dram_tensor("out", [N, C], F32, kind="ExternalOutput").ap()

    with tile.TileContext(nc) as tc:
        with ExitStack() as ctx:
            const = ctx.enter_context(tc.tile_pool(name="const", bufs=1))
            sb = ctx.enter_context(tc.tile_pool(name="sb", bufs=1))
            expp = ctx.enter_context(tc.tile_pool(name="expp", bufs=4))
            invp = ctx.enter_context(tc.tile_pool(name="invp", bufs=2))
            outp = ctx.enter_context(tc.tile_pool(name="outp", bufs=3))

            twq = const.tile([128, 512], BF16)
            nc.sync.dma_start(twq[:], wq[:])
            twk = const.tile([128, 512], BF16)
            nc.sync.dma_start(twk[:], wk[:])
            twv = const.tile([128, 512], BF16)
            nc.sync.dma_start(twv[:], wv[:])
            twp = const.tile([64, 1024], BF16)
            nc.sync.dma_start(twp[:], wp[:])
            tbias = const.tile([1, 512], BF16)
            nc.sync.dma_start(tbias[:], bias[:])
            tones = const.tile([1, 128], BF16)
            nc.gpsimd.memset(tones[:], 1.0)
            tones32 = const.tile([1, 64], F32)
            nc.gpsimd.memset(tones32[:], 1.0)

            qT = sb.tile([128, N], BF16)
            kT = sb.tile([128, N], BF16)
            vaug = sb.tile([128, NI * VW], BF16)
            nc.gpsimd.memset(vaug[:], 1.0)
            outT0 = sb.tile([64, N], BF16)
            outT1 = sb.tile([64, N], BF16)
            outTs = (outT0, outT1)

            xtp = ctx.enter_context(tc.tile_pool(name="xtp", bufs=1))
            psS = ctx.enter_context(tc.tile_pool(name="psS", bufs=2, space="PSUM"))
            psAV = ctx.enter_context(tc.tile_pool(name="psAV", bufs=1, space="PSUM"))
            psT = ctx.enter_context(tc.tile_pool(name="psT", bufs=2, space="PSUM"))

            # ---- stage A: QKV projections ------------------------------
            # Emitted as deadline-scheduled tasks threaded into the first two
            # j-blocks' i-loops (the PE queue is strict FIFO; anything emitted
            # before the first score matmul delays the first exp).
            xts = []
            for k in range(4):
                t = xtp.tile([128, N], BF16, tag=f"xt{k}", name=f"xt{k}")
                xts.append(t)
            for col in range(4):
                for k in range(4):
                    cs = bass.ts(col, N // 4)
                    nc.sync.dma_start(xts[k][:, cs], xt[k][:, cs])

            def emit_qk(j8, which):
                s_ = bass.ts(j8, 512)
                w, dst = (twq, qT) if which == "q" else (twk, kT)
                ps = psT.tile([128, 512], F32, tag="t", name="psqk")
                for k in range(4):
                    nc.tensor.matmul(
                        ps[:], w[:, bass.ts(k, 128)], xts[k][:, s_],
                        start=(k == 0), stop=(k == 3),
                    )
                nc.vector.tensor_copy(dst[:, s_], ps[:])

            def emit_v(jj):
                ps = psT.tile([128, 128], F32, tag="t", name="psv")
                for k in range(4):
                    nc.tensor.matmul(
                        ps[:], xts[k][:, bass.ts(jj, 128)], twv[:, bass.ts(k, 128)],
                        start=(k == 0), stop=(k == 3),
                    )
                dst = vaug[:, jj * VW : (jj + 1) * VW].rearrange(
                    "p (h c) -> p h c", h=2
                )[:, :, 0:D]
                src = ps[:].rearrange("p (h c) -> p h c", h=2)
                nc.vector.tensor_copy(dst, src)

            # (deadline in global i-steps, emitter) — qk k-chunk c feeds
            # scores at step 4c; v chunk jj feeds the AV matmul at step jj;
            # qk q-chunk j8 feeds block j8 (step 32*j8).
            stage_a_tasks = []
            for c in range(1, 8):
                stage_a_tasks.append((4 * c - 4, lambda c=c: emit_qk(c, "k")))
            for jj in range(1, NI):
                stage_a_tasks.append((jj - 2, lambda jj=jj: emit_v(jj)))
            for j8 in range(1, 8):
                stage_a_tasks.append((32 * j8 - 6, lambda j8=j8: emit_qk(j8, "q")))
            stage_a_tasks.sort(key=lambda t: t[0])
            stage_a_tasks = list(stage_a_tasks)

            # prologue: what step 0 needs
            emit_qk(0, "q")
            emit_qk(0, "k")
            emit_v(0)

            # ---- stage B: scores^T -> exp -> AV (+den), normalize -------
            # ---- stage C: partial projection + bias ---------------------
            # Tails (normalize + projection of block j) are emitted in the
            # middle of block j+1's i-loop: the PE queue is strict FIFO, so
            # matmuls that wait on the slow DVE reciprocal chain must sit
            # behind enough independent PE work to never stall the queue.
            def emit_bcast(st):
                h = st["h"]
                psb = psT.tile([64, 512], F32, tag="t", name="psb")
                nc.tensor.matmul(
                    psb[:], tones[0:1, 0:64], st["inv"][:], start=True, stop=True
                )
                sbb = invp.tile([64, 512], BF16, tag="sbb", name="sbb")
                nc.vector.tensor_copy(sbb[:], psb[:])
                nc.vector.tensor_mul(
                    outTs[h][:, st["qs"]], st["avsb"][0:64, :], sbb[:]
                )

            def emit_proj(j, k):
                jj = j * 4 + k
                s = bass.ts(jj, 128)
                pp = psT.tile([128, 512], F32, tag="t", name="pp")
                nc.tensor.matmul(
                    pp[:], outT0[:, s], twp[:, 0:512], start=True, stop=False
                )
                nc.tensor.matmul(
                    pp[:], outT1[:, s], twp[:, 512:1024], start=False, stop=False
                )
                nc.tensor.matmul(
                    pp[:], tones[0:1, :], tbias[:], start=False, stop=True
                )
                ot = outp.tile([128, 512], F32, tag="o", name="ot")
                nc.vector.tensor_copy(ot[:], pp[:])
                nc.sync.dma_start(out[s, :], ot[:])

            # Flat software pipeline over all 256 i-steps. AV matmuls are
            # emitted one step behind their scores/exp so the PE queue always
            # holds independent score work when an AV has to wait (block
            # boundary: the new accumulator bank frees only after the old
            # one's DVE evacuation).
            prev = None   # pending normalize/proj tail of the finished block
            pend = None   # (avs, est, start, stop, i) AV emission delayed 1 step
            avs = None
            NT = 8 * NI
            for gs in range(NT + 1):
                j, i = divmod(gs, NI)
                if gs < NT:
                    if i == 0:
                        avs = [
                            psAV.tile([65, 512], F32, tag=f"av{t}", name=f"av{t}")
                            for t in range(2)
                        ]
                    while stage_a_tasks and stage_a_tasks[0][0] <= gs + 3:
                        stage_a_tasks.pop(0)[1]()
                    if prev is not None:
                        if i == 6:
                            emit_bcast(prev["n"][0])
                        elif i == 10:
                            emit_bcast(prev["n"][1])
                        elif i >= 16 and i % 4 == 0:  # 16, 20, 24, 28
                            emit_proj(prev["j"], (i - 16) // 4)
                    qs = bass.ts(j, 512)
                    ks = bass.ts(i, 128)
                    pss = psS.tile([128, 1024], F32, tag="s")
                    nc.tensor.matmul(
                        pss[:, 0:512], kT[0:64, ks], qT[0:64, qs],
                        start=True, stop=True,
                    )
                    nc.tensor.matmul(
                        pss[:, 512:1024], kT[64:128, ks], qT[64:128, qs],
                        start=True, stop=True,
                    )
                    est = expp.tile([128, 1024], BF16, tag="e")
                    nc.scalar.activation(est[:], pss[:], AF.Exp, scale=SCALE)
                if pend is not None:
                    p_avs, p_est, p_start, p_stop, p_i, p_qs, p_j = pend
                    for h in range(2):
                        va = vaug[:, p_i * VW + h * 65 : p_i * VW + (h + 1) * 65]
                        nc.tensor.matmul(
                            p_avs[h][:], va, p_est[:, bass.ts(h, 512)],
                            start=p_start, stop=p_stop,
                        )
                    if p_stop:
                        # evacuate accumulators fast (releases banks for the
                        # new block) and start the reciprocals; the rest of
                        # the tail goes through the i==6/10/16+ hooks above.
                        norms = []
                        for h in range(2):
                            avsb = invp.tile(
                                [65, 512], F32, tag="avsb", name="avsb"
                            )
                            nc.vector.tensor_copy(avsb[:], p_avs[h][:])
                            inv = invp.tile([1, 512], BF16, tag="inv", name="inv")
                            with nc.allow_low_precision(
                                reason="softmax 1/den in bf16"
                            ):
                                nc.vector.reciprocal(inv[:], avsb[64:65, :])
                            norms.append(
                                {"h": h, "qs": p_qs, "avsb": avsb, "inv": inv}
                            )
                        prev = {"j": p_j, "n": norms}
                if gs < NT:
                    pend = (avs, est, i == 0, i == NI - 1, i, bass.ts(j, 512), j)
            # final block's tail
            for st in prev["n"]:
                emit_bcast(st)
            for k in range(4):
                emit_proj(prev["j"], k)

    nc.compile()
    return nc


def _pack_w(wslice):
    # [512, 128] -> SBUF image [128, 4*128] with C-chunk k at cols k*128..
    return np.ascontiguousarray(
        wslice.reshape(4, 128, 128).transpose(1, 0, 2).reshape(128, 512)
    ).astype(BFNP)


_NC_CACHE = None
LAST_RESULT = None


def kernel(x, W_qkv, W_proj, b_proj):
    global _NC_CACHE, LAST_RESULT
    x = np.asarray(x, dtype=np.float32)
    W_qkv = np.asarray(W_qkv, dtype=np.float32)
    W_proj = np.asarray(W_proj, dtype=np.float32)
    b_proj = np.asarray(b_proj, dtype=np.float32)

    if _NC_CACHE is None:
        _NC_CACHE = build_nc()
    nc = _NC_CACHE

    in_maps = []
    for c in range(N_CORES):
        b = c // 4
        h0 = 2 * (c % 4)
        xtb = np.ascontiguousarray(x[b].T).reshape(4, 128, N).astype(BFNP)
        wq = _pack_w(W_qkv[:, h0 * 64 : h0 * 64 + 128])
        wk = _pack_w(W_qkv[:, 512 + h0 * 64 : 512 + h0 * 64 + 128])
        wv = _pack_w(W_qkv[:, 1024 + h0 * 64 : 1024 + h0 * 64 + 128])
        wp = np.ascontiguousarray(
            np.concatenate(
                [
                    W_proj[h0 * 64 : (h0 + 1) * 64, :],
                    W_proj[(h0 + 1) * 64 : (h0 + 2) * 64, :],
                ],
                axis=1,
            )
        ).astype(BFNP)
        bias = (
            b_proj[None, :].astype(BFNP)
            if c % 4 == 0
            else np.zeros((1, 512), dtype=BFNP)
        )
        in_maps.append(
            {"xt": xtb, "wq": wq, "wk": wk, "wv": wv, "wp": wp, "bias": bias}
        )

    res = bass_utils.run_bass_kernel_spmd(
        nc, in_maps, core_ids=list(range(N_CORES))
    )
    LAST_RESULT = res

    out = np.zeros((B, N, C), dtype=np.float32)
    for c in range(N_CORES):
        out[c // 4] += res.results[c]["out"]
    return out


# revision 19
# speedup vs baseline: 1.5398x; 1.1142x over previous
"""Multi-head attention (B=2, N=4096, C=512, H=8, d=64) on 8 Trainium2 NeuronCores.

Sharding: core c handles batch b = c//4 and heads {2*(c%4), 2*(c%4)+1}.
Each core computes its 2 heads' attention plus a partial output projection
(contraction over its 128 rows of W_proj); the host gather sums the 4
partials per batch (bias is added on the p==0 core of each batch).

On-device dataflow (transposed-scores formulation, no on-chip transposes):
  qT/kT [128=2*64 d-dims, 4096]  = W.T @ x.T      (x.T supplied by host)
  v_aug [128 n-chunk, 32*(65+65)] = x @ Wv with a ones column per head
  S^T[kidx, q] = kT.T_chunk @ qT  (two heads ride row-groups 0-1 / 2-3
                                   of the PE array concurrently, K=64 each)
  E = exp(S^T / 8)                (ScalarE, scale folded into the LUT affine)
  [out_unnorm^T; den] = v_aug.T @ E   (ones column makes row 64 the softmax
                                       denominator -- no extra pass)
  out^T = out_unnorm^T * (1/den)  (reciprocal + K=1 broadcast matmul)
  partial = out^T.T @ W_proj_slice + bias   (per-head K=64 contractions)
"""

import sys
import types

for _p in ("/opt/trn_rl_repo",):
    if _p not in sys.path:
        sys.path.insert(0, _p)

import numpy as np
import ml_dtypes
from contextlib import ExitStack

# antenv.axon_hooks shim: lets run_bass_kernel_spmd find the NTFF profiling
# hook when BASS_TRACE=1 (the agent image's antenv lacks this module).
import antenv  # noqa: F401

if "antenv.axon_hooks" not in sys.modules:
    _m = types.ModuleType("antenv.axon_hooks")
    _m._hook = None

    def _set_hook(h):
        _m._hook = h

    def _get_hook():
        return _m._hook

    _m.set_axon_ntff_profile_hook = _set_hook
    _m.get_axon_ntff_profile_hook = _get_hook
    sys.modules["antenv.axon_hooks"] = _m
    try:
        from trn_agent_boot.trn_boot import _ntff_profile_via_ctypes

        hook = _ntff_profile_via_ctypes("/opt/axon/libaxon_pjrt.so")
        if hook is not None:
            _set_hook(hook)
    except Exception:
        pass

import concourse.bass as bass  # noqa: E402
import concourse.tile as tile  # noqa: E402
from concourse.tile import add_dep_helper  # noqa: E402
from concourse import mybir, bacc  # noqa: E402
from concourse import bass_utils  # noqa: E402

# No bucket storage in this container; artifacts stay local.
bass_utils.upload_artifacts = lambda tmpdir: f"local://{tmpdir}"

B, N, C = 2, 4096, 512
H, D = 8, 64
N_CORES = 8
SCALE = D ** -0.5

BF16 = mybir.dt.bfloat16
F32 = mybir.dt.float32
AF = mybir.ActivationFunctionType
BFNP = ml_dtypes.bfloat16

NI = N // 128   # 32 kidx / n chunks
NJ = N // 1024  # 4 q blocks
VW = 2 * (D + 1)  # 130: per-n-chunk vaug block (2 heads x (64 v + 1 ones))


def build_nc():
    nc = bacc.Bacc("TRN2", target_bir_lowering=False, debug=False)

    xt = nc.dram_tensor("xt", [4, 128, N], BF16, kind="ExternalInput").ap()
    wq = nc.dram_tensor("wq", [128, 512], BF16, kind="ExternalInput").ap()
    wk = nc.dram_tensor("wk", [128, 512], BF16, kind="ExternalInput").ap()
    wv = nc.dram_tensor("wv", [128, 512], BF16, kind="ExternalInput").ap()
    wp = nc.dram_tensor("wp", [64, 1024], BF16, kind="ExternalInput").ap()
    bias = nc.dram_tensor("bias", [1, 512], BF16, kind="ExternalInput").ap()
    out = nc.dram_tensor("out", [N, C], F32, kind="ExternalOutput").ap()

    with tile.TileContext(nc) as tc:
        with ExitStack() as ctx:
            const = ctx.enter_context(tc.tile_pool(name="const", bufs=1))
            sb = ctx.enter_context(tc.tile_pool(name="sb", bufs=1))
            expp = ctx.enter_context(tc.tile_pool(name="expp", bufs=4))
            invp = ctx.enter_context(tc.tile_pool(name="invp", bufs=2))
            outp = ctx.enter_context(tc.tile_pool(name="outp", bufs=3))

            twq = const.tile([128, 512], BF16)
            nc.sync.dma_start(twq[:], wq[:])
            twk = const.tile([128, 512], BF16)
            nc.sync.dma_start(twk[:], wk[:])
            twv = const.tile([128, 512], BF16)
            nc.sync.dma_start(twv[:], wv[:])
            twp = const.tile([64, 1024], BF16)
            nc.sync.dma_start(twp[:], wp[:])
            tbias = const.tile([1, 512], BF16)
            nc.sync.dma_start(tbias[:], bias[:])
            tones = const.tile([1, 128], BF16)
            nc.gpsimd.memset(tones[:], 1.0)
            tones32 = const.tile([1, 64], F32)
            nc.gpsimd.memset(tones32[:], 1.0)

            qT = sb.tile([128, N], BF16)
            kT = sb.tile([128, N], BF16)
            vaug = sb.tile([128, NI * VW], BF16)
            nc.gpsimd.memset(vaug[:], 1.0)
            outT0 = sb.tile([64, N], BF16)
            outT1 = sb.tile([64, N], BF16)
            outTs = (outT0, outT1)

            xtp = ctx.enter_context(tc.tile_pool(name="xtp", bufs=1))
            psS = ctx.enter_context(tc.tile_pool(name="psS", bufs=2, space="PSUM"))
            psAV = ctx.enter_context(tc.tile_pool(name="psAV", bufs=1, space="PSUM"))
            psT = ctx.enter_context(tc.tile_pool(name="psT", bufs=2, space="PSUM"))

            # ---- stage A: QKV projections ------------------------------
            # Emitted as deadline-scheduled tasks threaded into the first two
            # j-blocks' i-loops (the PE queue is strict FIFO; anything emitted
            # before the first score matmul delays the first exp).
            xts = []
            for k in range(4):
                t = xtp.tile([128, N], BF16, tag=f"xt{k}", name=f"xt{k}")
                xts.append(t)
            for col in range(4):
                for k in range(4):
                    cs = bass.ts(col, N // 4)
                    nc.sync.dma_start(xts[k][:, cs], xt[k][:, cs])

            def emit_qk(j8, which):
                s_ = bass.ts(j8, 512)
                w, dst = (twq, qT) if which == "q" else (twk, kT)
                ps = psT.tile([128, 512], F32, tag="t", name="psqk")
                for k in range(4):
                    nc.tensor.matmul(
                        ps[:], w[:, bass.ts(k, 128)], xts[k][:, s_],
                        start=(k == 0), stop=(k == 3),
                    )
                nc.vector.tensor_copy(dst[:, s_], ps[:])

            def emit_v(jj):
                ps = psT.tile([128, 128], F32, tag="t", name="psv")
                for k in range(4):
                    nc.tensor.matmul(
                        ps[:], xts[k][:, bass.ts(jj, 128)], twv[:, bass.ts(k, 128)],
                        start=(k == 0), stop=(k == 3),
                    )
                dst = vaug[:, jj * VW : (jj + 1) * VW].rearrange(
                    "p (h c) -> p h c", h=2
                )[:, :, 0:D]
                src = ps[:].rearrange("p (h c) -> p h c", h=2)
                nc.vector.tensor_copy(dst, src)

            # (deadline in global i-steps, emitter) — qk k-chunk c feeds
            # scores at step 4c; v chunk jj feeds the AV matmul at step jj;
            # qk q-chunk j8 feeds block j8 (step 32*j8).
            stage_a_tasks = []
            for c in range(1, 8):
                stage_a_tasks.append((4 * c - 4, lambda c=c: emit_qk(c, "k")))
            for jj in range(1, NI):
                stage_a_tasks.append((jj - 2, lambda jj=jj: emit_v(jj)))
            for j8 in range(1, 8):
                stage_a_tasks.append((32 * j8 - 6, lambda j8=j8: emit_qk(j8, "q")))
            stage_a_tasks.sort(key=lambda t: t[0])
            stage_a_tasks = list(stage_a_tasks)

            # prologue: what step 0 needs
            emit_qk(0, "q")
            emit_qk(0, "k")
            emit_v(0)

            # ---- stage B: scores^T -> exp -> AV (+den), normalize -------
            # ---- stage C: partial projection + bias ---------------------
            # Tails (normalize + projection of block j) are emitted in the
            # middle of block j+1's i-loop: the PE queue is strict FIFO, so
            # matmuls that wait on the slow DVE reciprocal chain must sit
            # behind enough independent PE work to never stall the queue.
            def emit_bcast(st, after=None):
                h = st["h"]
                psb = psT.tile([64, 512], F32, tag="t", name="psb")
                mi = nc.tensor.matmul(
                    psb[:], tones[0:1, 0:64], st["inv"][:], start=True, stop=True
                )
                if after is not None:
                    add_dep_helper(mi.ins, after.ins, sync=False,
                                   reason="tail behind scores")
                sbb = invp.tile([64, 512], BF16, tag="sbb", name="sbb")
                nc.vector.tensor_copy(sbb[:], psb[:])
                nc.vector.tensor_mul(
                    outTs[h][:, st["qs"]], st["avsb"][0:64, :], sbb[:]
                )

            def emit_proj(j, k, after=None):
                jj = j * 4 + k
                s = bass.ts(jj, 128)
                pp = psT.tile([128, 512], F32, tag="t", name="pp")
                mi = nc.tensor.matmul(
                    pp[:], outT0[:, s], twp[:, 0:512], start=True, stop=False
                )
                if after is not None:
                    add_dep_helper(mi.ins, after.ins, sync=False,
                                   reason="tail behind scores")
                nc.tensor.matmul(
                    pp[:], outT1[:, s], twp[:, 512:1024], start=False, stop=False
                )
                nc.tensor.matmul(
                    pp[:], tones[0:1, :], tbias[:], start=False, stop=True
                )
                ot = outp.tile([128, 512], F32, tag="o", name="ot")
                nc.vector.tensor_copy(ot[:], pp[:])
                nc.sync.dma_start(out[s, :], ot[:])

            # Flat software pipeline over all 256 i-steps. AV matmuls are
            # emitted one step behind their scores/exp so the PE queue always
            # holds independent score work when an AV has to wait (block
            # boundary: the new accumulator bank frees only after the old
            # one's DVE evacuation).
            prev = None   # pending normalize/proj tail of the finished block
            pend = None   # (avs, est, start, stop, i) AV emission delayed 1 step
            avs = None
            NT = 8 * NI
            for gs in range(NT + 1):
                j, i = divmod(gs, NI)
                if gs < NT:
                    if i == 0:
                        avs = [
                            psAV.tile([65, 512], F32, tag=f"av{t}", name=f"av{t}")
                            for t in range(2)
                        ]
                    while stage_a_tasks and stage_a_tasks[0][0] <= gs + 3:
                        stage_a_tasks.pop(0)[1]()
                    if prev is not None:
                        if i == 6:
                            emit_bcast(prev["n"][0], after=last_sc)
                        elif i == 10:
                            emit_bcast(prev["n"][1], after=last_sc)
                        elif i >= 16 and i % 4 == 0:  # 16, 20, 24, 28
                            emit_proj(prev["j"], (i - 16) // 4, after=last_sc)
                    qs = bass.ts(j, 512)
                    ks = bass.ts(i, 128)
                    pss = psS.tile([128, 1024], F32, tag="s")
                    nc.tensor.matmul(
                        pss[:, 0:512], kT[0:64, ks], qT[0:64, qs],
                        start=True, stop=True,
                    )
                    last_sc = nc.tensor.matmul(
                        pss[:, 512:1024], kT[64:128, ks], qT[64:128, qs],
                        start=True, stop=True,
                    )
                    est = expp.tile([128, 1024], BF16, tag="e")
                    nc.scalar.activation(est[:], pss[:], AF.Exp, scale=SCALE)
                if pend is not None:
                    p_avs, p_est, p_start, p_stop, p_i, p_qs, p_j = pend
                    for h in range(2):
                        va = vaug[:, p_i * VW + h * 65 : p_i * VW + (h + 1) * 65]
                        nc.tensor.matmul(
                            p_avs[h][:], va, p_est[:, bass.ts(h, 512)],
                            start=p_start, stop=p_stop,
                        )
                    if p_stop:
                        # evacuate accumulators fast (releases banks for the
                        # new block) and start the reciprocals; the rest of
                        # the tail goes through the i==6/10/16+ hooks above.
                        norms = []
                        for h in range(2):
                            avsb = invp.tile(
                                [65, 512], F32, tag="avsb", name="avsb"
                            )
                            nc.vector.tensor_copy(avsb[:], p_avs[h][:])
                            inv = invp.tile([1, 512], BF16, tag="inv", name="inv")
                            with nc.allow_low_precision(
                                reason="softmax 1/den in bf16"
                            ):
                                nc.vector.reciprocal(inv[:], avsb[64:65, :])
                            norms.append(
                                {"h": h, "qs": p_qs, "avsb": avsb, "inv": inv}
                            )
                        prev = {"j": p_j, "n": norms}
                if gs < NT:
                    pend = (avs, est, i == 0, i == NI - 1, i, bass.ts(j, 512), j)
            # final block's tail
            for st in prev["n"]:
                emit_bcast(st)
            for k in range(4):
                emit_proj(prev["j"], k)

    nc.compile()
    return nc


def _pack_w(wslice):
    # [512, 128] -> SBUF image [128, 4*128] with C-chunk k at cols k*128..
    return np.ascontiguousarray(
        wslice.reshape(4, 128, 128).transpose(1, 0, 2).reshape(128, 512)
    ).astype(BFNP)


_NC_CACHE = None
LAST_RESULT = None


def kernel(x, W_qkv, W_proj, b_proj):
    global _NC_CACHE, LAST_RESULT
    x = np.asarray(x, dtype=np.float32)
    W_qkv = np.asarray(W_qkv, dtype=np.float32)
    W_proj = np.asarray(W_proj, dtype=np.float32)
    b_proj = np.asarray(b_proj, dtype=np.float32)

    if _NC_CACHE is None:
        _NC_CACHE = build_nc()
    nc = _NC_CACHE

    in_maps = []
    for c in range(N_CORES):
        b = c // 4
        h0 = 2 * (c % 4)
        xtb = np.ascontiguousarray(x[b].T).reshape(4, 128, N).astype(BFNP)
        wq = _pack_w(W_qkv[:, h0 * 64 : h0 * 64 + 128])
        wk = _pack_w(W_qkv[:, 512 + h0 * 64 : 512 + h0 * 64 + 128])
        wv = _pack_w(W_qkv[:, 1024 + h0 * 64 : 1024 + h0 * 64 + 128])
        wp = np.ascontiguousarray(
            np.concatenate(
                [
                    W_proj[h0 * 64 : (h0 + 1) * 64, :],
                    W_proj[(h0 + 1) * 64 : (h0 + 2) * 64, :],
                ],
                axis=1,
            )
        ).astype(BFNP)
        bias = (
            b_proj[None, :].astype(BFNP)
            if c % 4 == 0
            else np.zeros((1, 512), dtype=BFNP)
        )
        in_maps.append(
            {"xt": xtb, "wq": wq, "wk": wk, "wv": wv, "wp": wp, "bias": bias}
        )

    res = bass_utils.run_bass_kernel_spmd(
        nc, in_maps, core_ids=list(range(N_CORES))
    )
    LAST_RESULT = res

    out = np.zeros((B, N, C), dtype=np.float32)
    for c in range(N_CORES):
        out[c // 4] += res.results[c]["out"]
    return out
